# revision 43
# baseline (speedup 1.0000x reference)
"""MoE FFN (top-1 switch routing) on 8 Trainium2 NeuronCores.

Strategy: dual-segment expert parallelism. Each core runs a fixed-size
main segment (CA tokens of its own expert) plus a spill segment (CB
tokens of any overloaded expert, with a second weight set), so the
per-core token capacity is CA+CB=536 instead of max-expert-count=608.
The router runs entirely on the host (it is needed for dispatch anyway);
the top-1 probability scale and b2 bias are applied during the host
scatter, so the device is a pure 2-matmul FFN:

    hT = relu(W1^T xg^T + b1)   (mlp on partitions, tokens moving)
    yT = W2^T hT                (d_model on partitions, tokens moving)

Both matmuls keep tokens as the moving dim => PE cycles ~ 512 * tokens
per core. Weights stream in host-packed layouts where every DMA is a
contiguous [128, 4096] block.
"""
import os
import sys
import numpy as np
import ml_dtypes

sys.path.insert(0, "/root/.axon_site")

import concourse.bass as bass
import concourse.bacc as bacc
import concourse.mybir as mybir
import concourse.tile as tile
import concourse.bass_utils as bass_utils

P = 128
D = 1024
MLP = 4096
E = 8
B, T = 4, 1024
N_TOK = B * T
KD = D // P      # 8 k-tiles over d_model
KM = MLP // P    # 32 k-tiles over mlp
MBC = 512        # W1 block = 512 mlp cols (4 m-tiles)
F32 = mybir.dt.float32
MM = mybir.dt.bfloat16
NP_MM = ml_dtypes.bfloat16
ADD = mybir.AluOpType.add
MAX = mybir.AluOpType.max
SPIN = int(os.environ.get("BASS_MOE_SPIN", "15"))

_cached_nc = {}
_host_cache = {}


def build_nc(CA, CB):
    nc = bacc.Bacc("TRN2", target_bir_lowering=False, debug=False)

    xga_d = nc.declare_dram_parameter("xga", [P, KD * CA], MM, isOutput=False)
    w1a_d = nc.declare_dram_parameter("w1a", [D, MLP], MM, isOutput=False)
    w2a_d = nc.declare_dram_parameter("w2a", [D, MLP], MM, isOutput=False)
    b1c_d = nc.declare_dram_parameter("b1c", [P, 2 * KM], F32, isOutput=False)
    ya_d = nc.declare_dram_parameter("ya", [D, CA], MM, isOutput=True)
    if CB:
        xgb_d = nc.declare_dram_parameter("xgb", [P, KD * CB], MM, isOutput=False)
        w1b_d = nc.declare_dram_parameter("w1b", [D, MLP], MM, isOutput=False)
        w2b_d = nc.declare_dram_parameter("w2b", [D, MLP], MM, isOutput=False)
        yb_d = nc.declare_dram_parameter("yb", [D, CB], MM, isOutput=True)

    with tile.TileContext(nc) as tc:
        with (
            tc.tile_pool(name="const", bufs=1) as cpool,
            tc.tile_pool(name="hpool", bufs=1) as hpool,
            tc.tile_pool(name="wap", bufs=5) as wap,
            tc.tile_pool(name="wbp", bufs=5) as wbp,
            tc.tile_pool(name="yout", bufs=4) as ypool,
        ):
            # ---- input DMAs head the scalar HWDGE ring (A weights ride sync,
            # B weights follow on scalar) ----
            half = KD // 2
            xga = cpool.tile([P, KD, CA], MM, tag="xga")
            nc.scalar.dma_start(out=xga[:, 0:half, :], in_=xga_d[:, 0:half * CA])
            nc.scalar.dma_start(out=xga[:, half:KD, :], in_=xga_d[:, half * CA:])
            if CB:
                xgb = cpool.tile([P, KD, CB], MM, tag="xgb")
                nc.scalar.dma_start(out=xgb[:], in_=xgb_d[:])
            b1c = cpool.tile([P, 2 * KM], F32, tag="b1c")
            nc.scalar.dma_start(out=b1c[:], in_=b1c_d[:])

            hta = hpool.tile([P, KM, CA], MM, tag="hta")
            if CB:
                htb = hpool.tile([P, KM, CB], MM, tag="htb")

            # ---- PE warm-up: spin matmuls on a zeroed tile while the first
            # input DMAs land, so the HAM clock gate is 8/8 when real work
            # starts. ----
            wsrc = cpool.tile([P, 512], MM, tag="wsrc")
            nc.vector.memset(wsrc[:], 0.0)
            with tc.tile_pool(name="ps_w", bufs=1, space="PSUM") as ps_w:
                wp = ps_w.tile([P, 512], F32, tag="warm")
                for i in range(SPIN):
                    nc.tensor.matmul(
                        wp[:], wsrc[:, 0:P], wsrc[:],
                        start=(i == 0), stop=(i == SPIN - 1),
                    )

            # A-segment chunks of <=512 moving tokens (PSUM bank limit)
            nach = -(-CA // 512)
            ach = []
            off = 0
            for i in range(nach):
                sz = (CA + nach - 1 - i) // nach
                ach.append((off, sz))
                off += sz

            # ---- FFN1: hT = relu(W1^T xg^T + b1) ----
            # W1 streams in 512-col blocks (contiguous 1MB DRAM regions). The
            # first B block rides the sync ring right behind A's so the B
            # stream isn't starved behind xg on the scalar ring; block 0 runs
            # all-A-then-all-B to match its later arrival.
            def ffn1(ps, mb):
                w1at = wap.tile([P, KD, MBC], MM, tag="w1at")
                if mb == 0:  # split first block so m-tile 0 starts sooner
                    nc.sync.dma_start(
                        out=w1at[:, 0:half, :], in_=w1a_d[0:P, 0:half * MBC])
                    nc.sync.dma_start(
                        out=w1at[:, half:KD, :], in_=w1a_d[0:P, half * MBC:])
                else:
                    nc.sync.dma_start(
                        out=w1at[:], in_=w1a_d[mb * P:(mb + 1) * P, :])
                if CB:
                    w1bt = wbp.tile([P, KD, MBC], MM, tag="w1bt")
                    if mb == 0:
                        nc.sync.dma_start(
                            out=w1bt[:, 0:half, :], in_=w1b_d[0:P, 0:half * MBC])
                        nc.sync.dma_start(
                            out=w1bt[:, half:KD, :], in_=w1b_d[0:P, half * MBC:])
                    else:
                        nc.scalar.dma_start(
                            out=w1bt[:], in_=w1b_d[mb * P:(mb + 1) * P, :])

                def a_tile(ml):
                    mt = mb * (MBC // P) + ml
                    pcs = [
                        ps.tile([P, sz], F32, tag=f"psA{ci}",
                                name=f"psA{mt}_{ci}")
                        for ci, (o, sz) in enumerate(ach)
                    ]
                    for k in range(KD):
                        for ci, (o, sz) in enumerate(ach):
                            nc.tensor.matmul(
                                pcs[ci][:],
                                w1at[:, k, ml * P:(ml + 1) * P],
                                xga[:, k, o:o + sz],
                                start=(k == 0), stop=(k == KD - 1),
                            )
                    for ci, (o, sz) in enumerate(ach):
                        nc.vector.tensor_scalar(
                            hta[:, mt, o:o + sz], pcs[ci][:],
                            b1c[:, mt:mt + 1], 0.0, ADD, MAX)

                def b_tile(ml):
                    mt = mb * (MBC // P) + ml
                    psB = ps.tile([P, CB], F32, tag="psB", name=f"psB{mt}")
                    for k in range(KD):
                        nc.tensor.matmul(
                            psB[:],
                            w1bt[:, k, ml * P:(ml + 1) * P],
                            xgb[:, k, :],
                            start=(k == 0), stop=(k == KD - 1),
                        )
                    nc.vector.tensor_scalar(
                        htb[:, mt, :], psB[:],
                        b1c[:, KM + mt:KM + mt + 1], 0.0, ADD, MAX)

                if CB and mb == 0:
                    for ml in range(MBC // P):
                        a_tile(ml)
                    for ml in range(MBC // P):
                        b_tile(ml)
                else:
                    # interleave A/B per k so each LDWEIGHTS hides behind the
                    # previous (longer) A matmul
                    for ml in range(MBC // P):
                        mt = mb * (MBC // P) + ml
                        pcs = [
                            ps.tile([P, sz], F32, tag=f"psA{ci}",
                                    name=f"psA{mt}_{ci}")
                            for ci, (o, sz) in enumerate(ach)
                        ]
                        if CB:
                            psB = ps.tile([P, CB], F32, tag="psB",
                                          name=f"psB{mt}")
                        for k in range(KD):
                            for ci, (o, sz) in enumerate(ach):
                                nc.tensor.matmul(
                                    pcs[ci][:],
                                    w1at[:, k, ml * P:(ml + 1) * P],
                                    xga[:, k, o:o + sz],
                                    start=(k == 0), stop=(k == KD - 1),
                                )
                            if CB:
                                nc.tensor.matmul(
                                    psB[:],
                                    w1bt[:, k, ml * P:(ml + 1) * P],
                                    xgb[:, k, :],
                                    start=(k == 0), stop=(k == KD - 1),
                                )
                        for ci, (o, sz) in enumerate(ach):
                            nc.vector.tensor_scalar(
                                hta[:, mt, o:o + sz], pcs[ci][:],
                                b1c[:, mt:mt + 1], 0.0, ADD, MAX)
                        if CB:
                            nc.vector.tensor_scalar(
                                htb[:, mt, :], psB[:],
                                b1c[:, KM + mt:KM + mt + 1], 0.0, ADD, MAX)

            with tc.tile_pool(name="ps1", bufs=3, space="PSUM") as ps1:
                for mb in range(MLP // MBC):
                    ffn1(ps1, mb)

            # ---- FFN2: yT = W2^T hT ----
            def ffn2(ps, dt):
                w2at = wap.tile([P, KM, P], MM, tag="w2at")
                nc.sync.dma_start(out=w2at[:], in_=w2a_d[dt * P:(dt + 1) * P, :])
                if CB:
                    w2bt = wbp.tile([P, KM, P], MM, tag="w2bt")
                    nc.scalar.dma_start(
                        out=w2bt[:], in_=w2b_d[dt * P:(dt + 1) * P, :])
                pcs = [
                    ps.tile([P, sz], F32, tag=f"ps2A{ci}", name=f"ps2A{dt}_{ci}")
                    for ci, (o, sz) in enumerate(ach)
                ]
                if CB:
                    psB = ps.tile([P, CB], F32, tag="ps2B", name=f"ps2B{dt}")
                for k in range(KM):
                    for ci, (o, sz) in enumerate(ach):
                        nc.tensor.matmul(
                            pcs[ci][:], w2at[:, k, :], hta[:, k, o:o + sz],
                            start=(k == 0), stop=(k == KM - 1),
                        )
                    if CB:
                        nc.tensor.matmul(
                            psB[:], w2bt[:, k, :], htb[:, k, :],
                            start=(k == 0), stop=(k == KM - 1),
                        )
                yat = ypool.tile([P, CA], MM, tag="yat")
                for ci, (o, sz) in enumerate(ach):
                    nc.vector.tensor_copy(yat[:, o:o + sz], pcs[ci][:])
                # y rides the (otherwise idle) gpsimd SWDGE queue; the last
                # d-tile goes out on sync/scalar (empty by then) for a short tail
                eng_a = nc.sync if dt == D // P - 1 else nc.gpsimd
                eng_a.dma_start(out=ya_d[dt * P:(dt + 1) * P, :], in_=yat[:])
                if CB:
                    ybt = ypool.tile([P, CB], MM, tag="ybt")
                    nc.vector.tensor_copy(ybt[:], psB[:])
                    eng_b = nc.scalar if dt == D // P - 1 else nc.gpsimd
                    eng_b.dma_start(out=yb_d[dt * P:(dt + 1) * P, :], in_=ybt[:])

            with tc.tile_pool(name="ps2", bufs=3, space="PSUM") as ps2:
                for dt in range(D // P):
                    ffn2(ps2, dt)
    nc.compile()
    return nc


def _choose_capacity(counts):
    """Pick (CA, CB) so all spill chunks fit in the 8 per-core B slots."""
    force = os.environ.get("BASS_MOE_FORCE")
    if force:
        CA, CB = (int(v) for v in force.split(","))
        if CA >= max(counts) or (
                CB and sum(-(-max(c - CA, 0) // CB) for c in counts) <= E):
            return CA, CB
    for CA, CB in [(464, 72), (512, 96), (576, 128), (640, 192)]:
        need = sum(-(-max(c - CA, 0) // CB) for c in counts)
        if need <= E:
            return CA, CB
    return max(counts), 0


def _pack_weights(W1, W2, b1):
    """Host-side DRAM layouts: every device DMA is a contiguous [128,N] block."""
    w1x = np.ascontiguousarray(
        W1.reshape(E, KD, P, MLP // MBC, MBC).transpose(0, 3, 2, 1, 4)
        .reshape(E, D, MLP)).astype(NP_MM)
    w2x = np.ascontiguousarray(
        W2.reshape(E, KM, P, KD, P).transpose(0, 3, 2, 1, 4)
        .reshape(E, D, MLP)).astype(NP_MM)
    b1x = np.ascontiguousarray(b1.reshape(E, KM, P).transpose(0, 2, 1))
    return w1x, w2x, b1x


def _xg_pack(x_rows, C):
    """[n,D] tokens -> [128, KD*C] bf16 (k-major, zero-padded)."""
    out = np.zeros((P, KD * C), NP_MM)
    n = x_rows.shape[0]
    if n:
        xt = x_rows.T.astype(NP_MM).reshape(KD, P, n).transpose(1, 0, 2)
        out.reshape(P, KD, C)[:, :, :n] = xt
    return out


def kernel(x, w_gate, b_gate, W1, b1, W2, b2):
    x = np.ascontiguousarray(x, np.float32)
    w_gate = np.ascontiguousarray(w_gate, np.float32)
    b_gate = np.ascontiguousarray(b_gate, np.float32)
    W1 = np.ascontiguousarray(W1, np.float32)
    b1 = np.ascontiguousarray(b1, np.float32)
    W2 = np.ascontiguousarray(W2, np.float32)
    b2 = np.ascontiguousarray(b2, np.float32)

    x_flat = x.reshape(N_TOK, D)
    logits = x_flat @ w_gate + b_gate
    idx = logits.argmax(-1)
    lmax = logits.max(-1, keepdims=True)
    ee = np.exp(logits - lmax)
    p = (ee.max(-1) / ee.sum(-1)).astype(np.float32)

    ids = [np.nonzero(idx == e)[0] for e in range(E)]
    counts = [len(i) for i in ids]
    CA, CB = _choose_capacity(counts)

    wkey = (W1.ctypes.data, W2.ctypes.data, b1.ctypes.data)
    if _host_cache.get("wkey") != wkey:
        _host_cache["wkey"] = wkey
        _host_cache["packed"] = _pack_weights(W1, W2, b1)
    w1x, w2x, b1x = _host_cache["packed"]

    # spill chunks: tokens beyond CA per expert, split into <=CB pieces
    chunks = []
    for e in range(E):
        rem = ids[e][CA:]
        for s in range(0, len(rem), CB if CB else 1):
            if CB:
                chunks.append((e, rem[s:s + CB]))
    assert len(chunks) <= E, (counts, CA, CB)

    in_maps = []
    b_assign = []
    for c in range(E):
        a_ids = ids[c][:CA]
        m = {
            "xga": _xg_pack(x_flat[a_ids], CA),
            "w1a": w1x[c], "w2a": w2x[c],
        }
        if c < len(chunks):
            eb, b_ids = chunks[c]
        else:
            eb, b_ids = c, np.empty(0, np.int64)
        b_assign.append((eb, b_ids))
        b1cm = np.concatenate([b1x[c], b1x[eb]], axis=1).astype(np.float32)
        m["b1c"] = np.ascontiguousarray(b1cm)
        if CB:
            m["xgb"] = _xg_pack(x_flat[b_ids], CB)
            m["w1b"] = w1x[eb]
            m["w2b"] = w2x[eb]
        in_maps.append(m)

    key = (CA, CB)
    if key not in _cached_nc:
        _cached_nc[key] = build_nc(CA, CB)
    nc = _cached_nc[key]

    res = bass_utils.run_bass_kernel_spmd(nc, in_maps, list(range(E)))

    out_flat = np.empty((N_TOK, D), np.float32)
    for c in range(E):
        a_ids = ids[c][:CA]
        ya = res.results[c]["ya"].astype(np.float32)  # [D, CA]
        out_flat[a_ids] = (ya.T[:len(a_ids)] + b2[c]) * p[a_ids, None]
        eb, b_ids = b_assign[c]
        if len(b_ids):
            yb = res.results[c]["yb"].astype(np.float32)
            out_flat[b_ids] = (yb.T[:len(b_ids)] + b2[eb]) * p[b_ids, None]
    return out_flat.reshape(B, T, D)


# revision 44
# speedup vs baseline: 1.0387x; 1.0387x over previous
"""MoE FFN (top-1 switch routing) on 8 Trainium2 NeuronCores.

Strategy: dual-segment expert parallelism. Each core runs a fixed-size
main segment (CA tokens of its own expert) plus a spill segment (CB
tokens of any overloaded expert, with a second weight set), so the
per-core token capacity is CA+CB=536 instead of max-expert-count=608.
The router runs entirely on the host (it is needed for dispatch anyway);
the top-1 probability scale and b2 bias are applied during the host
scatter, so the device is a pure 2-matmul FFN:

    hT = relu(W1^T xg^T + b1)   (mlp on partitions, tokens moving)
    yT = W2^T hT                (d_model on partitions, tokens moving)

Both matmuls keep tokens as the moving dim => PE cycles ~ 512 * tokens
per core. Weights stream in host-packed layouts where every DMA is a
contiguous [128, 4096] block.
"""
import os
import sys
import numpy as np
import ml_dtypes

sys.path.insert(0, "/root/.axon_site")

import concourse.bass as bass
import concourse.bacc as bacc
import concourse.mybir as mybir
import concourse.tile as tile
import concourse.bass_utils as bass_utils

P = 128
D = 1024
MLP = 4096
E = 8
B, T = 4, 1024
N_TOK = B * T
KD = D // P      # 8 k-tiles over d_model
KM = MLP // P    # 32 k-tiles over mlp
MBC = 512        # W1 block = 512 mlp cols (4 m-tiles)
F32 = mybir.dt.float32
MM = mybir.dt.bfloat16
NP_MM = ml_dtypes.bfloat16
ADD = mybir.AluOpType.add
MAX = mybir.AluOpType.max
SPIN = int(os.environ.get("BASS_MOE_SPIN", "15"))

_cached_nc = {}
_host_cache = {}


def build_nc(CA, CB):
    nc = bacc.Bacc("TRN2", target_bir_lowering=False, debug=False)

    xga_d = nc.declare_dram_parameter("xga", [P, KD * CA], MM, isOutput=False)
    w1a_d = nc.declare_dram_parameter("w1a", [D, MLP], MM, isOutput=False)
    w2a_d = nc.declare_dram_parameter("w2a", [D, MLP], MM, isOutput=False)
    b1c_d = nc.declare_dram_parameter("b1c", [P, 2 * KM], F32, isOutput=False)
    ya_d = nc.declare_dram_parameter("ya", [D, CA], MM, isOutput=True)
    if CB:
        xgb_d = nc.declare_dram_parameter("xgb", [P, KD * CB], MM, isOutput=False)
        w1b_d = nc.declare_dram_parameter("w1b", [D, MLP], MM, isOutput=False)
        w2b_d = nc.declare_dram_parameter("w2b", [D, MLP], MM, isOutput=False)
        yb_d = nc.declare_dram_parameter("yb", [D, CB], MM, isOutput=True)

    with tile.TileContext(nc) as tc:
        with (
            tc.tile_pool(name="const", bufs=1) as cpool,
            tc.tile_pool(name="hpool", bufs=1) as hpool,
            tc.tile_pool(name="wap", bufs=5) as wap,
            tc.tile_pool(name="wbp", bufs=5) as wbp,
            tc.tile_pool(name="yout", bufs=4) as ypool,
        ):
            # ---- input DMAs head the scalar HWDGE ring (A weights ride sync,
            # B weights follow on scalar) ----
            half = KD // 2
            xga = cpool.tile([P, KD, CA], MM, tag="xga")
            nc.scalar.dma_start(out=xga[:, 0:half, :], in_=xga_d[:, 0:half * CA])
            nc.scalar.dma_start(out=xga[:, half:KD, :], in_=xga_d[:, half * CA:])
            if CB:
                xgb = cpool.tile([P, KD, CB], MM, tag="xgb")
                nc.scalar.dma_start(out=xgb[:], in_=xgb_d[:])
            b1c = cpool.tile([P, 2 * KM], F32, tag="b1c")
            nc.scalar.dma_start(out=b1c[:], in_=b1c_d[:])

            hta = hpool.tile([P, KM, CA], MM, tag="hta")
            if CB:
                htb = hpool.tile([P, KM, CB], MM, tag="htb")

            # ---- PE warm-up: spin matmuls on a zeroed tile while the first
            # input DMAs land, so the HAM clock gate is 8/8 when real work
            # starts. ----
            wsrc = cpool.tile([P, 512], MM, tag="wsrc")
            nc.vector.memset(wsrc[:], 0.0)
            with tc.tile_pool(name="ps_w", bufs=1, space="PSUM") as ps_w:
                wp = ps_w.tile([P, 512], F32, tag="warm")
                for i in range(SPIN):
                    nc.tensor.matmul(
                        wp[:], wsrc[:, 0:P], wsrc[:],
                        start=(i == 0), stop=(i == SPIN - 1),
                    )

            # A-segment chunks of <=512 moving tokens (PSUM bank limit)
            nach = -(-CA // 512)
            ach = []
            off = 0
            for i in range(nach):
                sz = (CA + nach - 1 - i) // nach
                ach.append((off, sz))
                off += sz

            # ---- FFN1: hT = relu(W1^T xg^T + b1) ----
            # W1 streams in 512-col blocks (contiguous 1MB DRAM regions). The
            # first B block rides the sync ring right behind A's so the B
            # stream isn't starved behind xg on the scalar ring; block 0 runs
            # all-A-then-all-B to match its later arrival.
            def ffn1(ps, mb):
                # every block lands as two k-half DMAs: the consuming k-loop
                # unblocks on the first half's semaphore instead of waiting
                # for the whole 1MB block's completion (~2us earlier under the
                # observed completion-semaphore lag)
                w1at = wap.tile([P, KD, MBC], MM, tag="w1at")
                r = slice(mb * P, (mb + 1) * P)
                nc.sync.dma_start(
                    out=w1at[:, 0:half, :], in_=w1a_d[r, 0:half * MBC])
                nc.sync.dma_start(
                    out=w1at[:, half:KD, :], in_=w1a_d[r, half * MBC:])
                if CB:
                    w1bt = wbp.tile([P, KD, MBC], MM, tag="w1bt")
                    eng = nc.sync if mb == 0 else nc.scalar
                    eng.dma_start(
                        out=w1bt[:, 0:half, :], in_=w1b_d[r, 0:half * MBC])
                    eng.dma_start(
                        out=w1bt[:, half:KD, :], in_=w1b_d[r, half * MBC:])

                def a_tile(ml):
                    mt = mb * (MBC // P) + ml
                    pcs = [
                        ps.tile([P, sz], F32, tag=f"psA{ci}",
                                name=f"psA{mt}_{ci}")
                        for ci, (o, sz) in enumerate(ach)
                    ]
                    for k in range(KD):
                        for ci, (o, sz) in enumerate(ach):
                            nc.tensor.matmul(
                                pcs[ci][:],
                                w1at[:, k, ml * P:(ml + 1) * P],
                                xga[:, k, o:o + sz],
                                start=(k == 0), stop=(k == KD - 1),
                            )
                    for ci, (o, sz) in enumerate(ach):
                        nc.vector.tensor_scalar(
                            hta[:, mt, o:o + sz], pcs[ci][:],
                            b1c[:, mt:mt + 1], 0.0, ADD, MAX)

                def b_tile(ml):
                    mt = mb * (MBC // P) + ml
                    psB = ps.tile([P, CB], F32, tag="psB", name=f"psB{mt}")
                    for k in range(KD):
                        nc.tensor.matmul(
                            psB[:],
                            w1bt[:, k, ml * P:(ml + 1) * P],
                            xgb[:, k, :],
                            start=(k == 0), stop=(k == KD - 1),
                        )
                    nc.vector.tensor_scalar(
                        htb[:, mt, :], psB[:],
                        b1c[:, KM + mt:KM + mt + 1], 0.0, ADD, MAX)

                if CB and mb == 0:
                    for ml in range(MBC // P):
                        a_tile(ml)
                    for ml in range(MBC // P):
                        b_tile(ml)
                else:
                    # interleave A/B per k so each LDWEIGHTS hides behind the
                    # previous (longer) A matmul
                    for ml in range(MBC // P):
                        mt = mb * (MBC // P) + ml
                        pcs = [
                            ps.tile([P, sz], F32, tag=f"psA{ci}",
                                    name=f"psA{mt}_{ci}")
                            for ci, (o, sz) in enumerate(ach)
                        ]
                        if CB:
                            psB = ps.tile([P, CB], F32, tag="psB",
                                          name=f"psB{mt}")
                        for k in range(KD):
                            for ci, (o, sz) in enumerate(ach):
                                nc.tensor.matmul(
                                    pcs[ci][:],
                                    w1at[:, k, ml * P:(ml + 1) * P],
                                    xga[:, k, o:o + sz],
                                    start=(k == 0), stop=(k == KD - 1),
                                )
                            if CB:
                                nc.tensor.matmul(
                                    psB[:],
                                    w1bt[:, k, ml * P:(ml + 1) * P],
                                    xgb[:, k, :],
                                    start=(k == 0), stop=(k == KD - 1),
                                )
                        for ci, (o, sz) in enumerate(ach):
                            nc.vector.tensor_scalar(
                                hta[:, mt, o:o + sz], pcs[ci][:],
                                b1c[:, mt:mt + 1], 0.0, ADD, MAX)
                        if CB:
                            nc.vector.tensor_scalar(
                                htb[:, mt, :], psB[:],
                                b1c[:, KM + mt:KM + mt + 1], 0.0, ADD, MAX)

            with tc.tile_pool(name="ps1", bufs=3, space="PSUM") as ps1:
                for mb in range(MLP // MBC):
                    ffn1(ps1, mb)

            # ---- FFN2: yT = W2^T hT ----
            def ffn2(ps, dt):
                w2at = wap.tile([P, KM, P], MM, tag="w2at")
                nc.sync.dma_start(out=w2at[:], in_=w2a_d[dt * P:(dt + 1) * P, :])
                if CB:
                    w2bt = wbp.tile([P, KM, P], MM, tag="w2bt")
                    nc.scalar.dma_start(
                        out=w2bt[:], in_=w2b_d[dt * P:(dt + 1) * P, :])
                pcs = [
                    ps.tile([P, sz], F32, tag=f"ps2A{ci}", name=f"ps2A{dt}_{ci}")
                    for ci, (o, sz) in enumerate(ach)
                ]
                if CB:
                    psB = ps.tile([P, CB], F32, tag="ps2B", name=f"ps2B{dt}")
                for k in range(KM):
                    for ci, (o, sz) in enumerate(ach):
                        nc.tensor.matmul(
                            pcs[ci][:], w2at[:, k, :], hta[:, k, o:o + sz],
                            start=(k == 0), stop=(k == KM - 1),
                        )
                    if CB:
                        nc.tensor.matmul(
                            psB[:], w2bt[:, k, :], htb[:, k, :],
                            start=(k == 0), stop=(k == KM - 1),
                        )
                yat = ypool.tile([P, CA], MM, tag="yat")
                for ci, (o, sz) in enumerate(ach):
                    nc.vector.tensor_copy(yat[:, o:o + sz], pcs[ci][:])
                # y rides the (otherwise idle) gpsimd SWDGE queue; the last
                # d-tile goes out on sync/scalar (empty by then) for a short tail
                eng_a = nc.sync if dt == D // P - 1 else nc.gpsimd
                eng_a.dma_start(out=ya_d[dt * P:(dt + 1) * P, :], in_=yat[:])
                if CB:
                    ybt = ypool.tile([P, CB], MM, tag="ybt")
                    nc.vector.tensor_copy(ybt[:], psB[:])
                    eng_b = nc.scalar if dt == D // P - 1 else nc.gpsimd
                    eng_b.dma_start(out=yb_d[dt * P:(dt + 1) * P, :], in_=ybt[:])

            with tc.tile_pool(name="ps2", bufs=3, space="PSUM") as ps2:
                for dt in range(D // P):
                    ffn2(ps2, dt)
    nc.compile()
    return nc


def _choose_capacity(counts):
    """Pick (CA, CB) so all spill chunks fit in the 8 per-core B slots."""
    force = os.environ.get("BASS_MOE_FORCE")
    if force:
        CA, CB = (int(v) for v in force.split(","))
        if CA >= max(counts) or (
                CB and sum(-(-max(c - CA, 0) // CB) for c in counts) <= E):
            return CA, CB
    for CA, CB in [(464, 72), (512, 96), (576, 128), (640, 192)]:
        need = sum(-(-max(c - CA, 0) // CB) for c in counts)
        if need <= E:
            return CA, CB
    return max(counts), 0


def _pack_weights(W1, W2, b1):
    """Host-side DRAM layouts: every device DMA is a contiguous [128,N] block."""
    w1x = np.ascontiguousarray(
        W1.reshape(E, KD, P, MLP // MBC, MBC).transpose(0, 3, 2, 1, 4)
        .reshape(E, D, MLP)).astype(NP_MM)
    w2x = np.ascontiguousarray(
        W2.reshape(E, KM, P, KD, P).transpose(0, 3, 2, 1, 4)
        .reshape(E, D, MLP)).astype(NP_MM)
    b1x = np.ascontiguousarray(b1.reshape(E, KM, P).transpose(0, 2, 1))
    return w1x, w2x, b1x


def _xg_pack(x_rows, C):
    """[n,D] tokens -> [128, KD*C] bf16 (k-major, zero-padded)."""
    out = np.zeros((P, KD * C), NP_MM)
    n = x_rows.shape[0]
    if n:
        xt = x_rows.T.astype(NP_MM).reshape(KD, P, n).transpose(1, 0, 2)
        out.reshape(P, KD, C)[:, :, :n] = xt
    return out


def kernel(x, w_gate, b_gate, W1, b1, W2, b2):
    x = np.ascontiguousarray(x, np.float32)
    w_gate = np.ascontiguousarray(w_gate, np.float32)
    b_gate = np.ascontiguousarray(b_gate, np.float32)
    W1 = np.ascontiguousarray(W1, np.float32)
    b1 = np.ascontiguousarray(b1, np.float32)
    W2 = np.ascontiguousarray(W2, np.float32)
    b2 = np.ascontiguousarray(b2, np.float32)

    x_flat = x.reshape(N_TOK, D)
    logits = x_flat @ w_gate + b_gate
    idx = logits.argmax(-1)
    lmax = logits.max(-1, keepdims=True)
    ee = np.exp(logits - lmax)
    p = (ee.max(-1) / ee.sum(-1)).astype(np.float32)

    ids = [np.nonzero(idx == e)[0] for e in range(E)]
    counts = [len(i) for i in ids]
    CA, CB = _choose_capacity(counts)

    wkey = (W1.ctypes.data, W2.ctypes.data, b1.ctypes.data)
    if _host_cache.get("wkey") != wkey:
        _host_cache["wkey"] = wkey
        _host_cache["packed"] = _pack_weights(W1, W2, b1)
    w1x, w2x, b1x = _host_cache["packed"]

    # spill chunks: tokens beyond CA per expert, split into <=CB pieces
    chunks = []
    for e in range(E):
        rem = ids[e][CA:]
        for s in range(0, len(rem), CB if CB else 1):
            if CB:
                chunks.append((e, rem[s:s + CB]))
    assert len(chunks) <= E, (counts, CA, CB)

    in_maps = []
    b_assign = []
    for c in range(E):
        a_ids = ids[c][:CA]
        m = {
            "xga": _xg_pack(x_flat[a_ids], CA),
            "w1a": w1x[c], "w2a": w2x[c],
        }
        if c < len(chunks):
            eb, b_ids = chunks[c]
        else:
            eb, b_ids = c, np.empty(0, np.int64)
        b_assign.append((eb, b_ids))
        b1cm = np.concatenate([b1x[c], b1x[eb]], axis=1).astype(np.float32)
        m["b1c"] = np.ascontiguousarray(b1cm)
        if CB:
            m["xgb"] = _xg_pack(x_flat[b_ids], CB)
            m["w1b"] = w1x[eb]
            m["w2b"] = w2x[eb]
        in_maps.append(m)

    key = (CA, CB)
    if key not in _cached_nc:
        _cached_nc[key] = build_nc(CA, CB)
    nc = _cached_nc[key]

    res = bass_utils.run_bass_kernel_spmd(nc, in_maps, list(range(E)))

    out_flat = np.empty((N_TOK, D), np.float32)
    for c in range(E):
        a_ids = ids[c][:CA]
        ya = res.results[c]["ya"].astype(np.float32)  # [D, CA]
        out_flat[a_ids] = (ya.T[:len(a_ids)] + b2[c]) * p[a_ids, None]
        eb, b_ids = b_assign[c]
        if len(b_ids):
            yb = res.results[c]["yb"].astype(np.float32)
            out_flat[b_ids] = (yb.T[:len(b_ids)] + b2[eb]) * p[b_ids, None]
    return out_flat.reshape(B, T, D)


# revision 46
# speedup vs baseline: 1.0648x; 1.0252x over previous
"""MoE FFN (top-1 switch routing) on 8 Trainium2 NeuronCores.

Strategy: dual-segment expert parallelism. Each core runs a fixed-size
main segment (CA tokens of its own expert) plus a spill segment (CB
tokens of any overloaded expert, with a second weight set), so the
per-core token capacity is CA+CB=536 instead of max-expert-count=608.
The router runs entirely on the host (it is needed for dispatch anyway);
the top-1 probability scale and b2 bias are applied during the host
scatter, so the device is a pure 2-matmul FFN:

    hT = relu(W1^T xg^T + b1)   (mlp on partitions, tokens moving)
    yT = W2^T hT                (d_model on partitions, tokens moving)

Both matmuls keep tokens as the moving dim => PE cycles ~ 512 * tokens
per core. Weights stream in host-packed layouts where every DMA is a
contiguous [128, 4096] block.
"""
import os
import sys
import numpy as np
import ml_dtypes

sys.path.insert(0, "/root/.axon_site")

import concourse.bass as bass
import concourse.bacc as bacc
import concourse.mybir as mybir
import concourse.tile as tile
import concourse.bass_utils as bass_utils

P = 128
D = 1024
MLP = 4096
E = 8
B, T = 4, 1024
N_TOK = B * T
KD = D // P      # 8 k-tiles over d_model
KM = MLP // P    # 32 k-tiles over mlp
MBC = 512        # W1 block = 512 mlp cols (4 m-tiles)
F32 = mybir.dt.float32
MM = mybir.dt.bfloat16
NP_MM = ml_dtypes.bfloat16
ADD = mybir.AluOpType.add
MAX = mybir.AluOpType.max
SPIN = int(os.environ.get("BASS_MOE_SPIN", "15"))

_cached_nc = {}
_host_cache = {}


def build_nc(CA, CB):
    nc = bacc.Bacc("TRN2", target_bir_lowering=False, debug=False)

    xga_d = nc.declare_dram_parameter("xga", [P, KD * CA], MM, isOutput=False)
    w1a_d = nc.declare_dram_parameter("w1a", [D, MLP], MM, isOutput=False)
    w2a_d = nc.declare_dram_parameter("w2a", [D, MLP], MM, isOutput=False)
    b1c_d = nc.declare_dram_parameter("b1c", [P, 2 * KM], F32, isOutput=False)
    ya_d = nc.declare_dram_parameter("ya", [D, CA], MM, isOutput=True)
    if CB:
        xgb_d = nc.declare_dram_parameter("xgb", [P, KD * CB], MM, isOutput=False)
        w1b_d = nc.declare_dram_parameter("w1b", [D, MLP], MM, isOutput=False)
        w2b_d = nc.declare_dram_parameter("w2b", [D, MLP], MM, isOutput=False)
        yb_d = nc.declare_dram_parameter("yb", [D, CB], MM, isOutput=True)

    with tile.TileContext(nc) as tc:
        with (
            tc.tile_pool(name="const", bufs=1) as cpool,
            tc.tile_pool(name="hpool", bufs=1) as hpool,
            tc.tile_pool(name="wap", bufs=5) as wap,
            tc.tile_pool(name="wbp", bufs=5) as wbp,
            tc.tile_pool(name="yout", bufs=4) as ypool,
        ):
            # ---- input DMAs head the scalar HWDGE ring (A weights ride sync,
            # B weights follow on scalar) ----
            half = KD // 2
            xga = cpool.tile([P, KD, CA], MM, tag="xga")
            for q in range(4):  # k-quarters: m-tile 0's k-loop unblocks early
                nc.scalar.dma_start(
                    out=xga[:, 2 * q:2 * (q + 1), :],
                    in_=xga_d[:, 2 * q * CA:2 * (q + 1) * CA])
            if CB:
                xgb = cpool.tile([P, KD, CB], MM, tag="xgb")
                nc.scalar.dma_start(out=xgb[:], in_=xgb_d[:])
            b1c = cpool.tile([P, 2 * KM], F32, tag="b1c")
            nc.scalar.dma_start(out=b1c[:], in_=b1c_d[:])

            hta = hpool.tile([P, KM, CA], MM, tag="hta")
            if CB:
                htb = hpool.tile([P, KM, CB], MM, tag="htb")

            # ---- PE warm-up: spin matmuls on a zeroed tile while the first
            # input DMAs land, so the HAM clock gate is 8/8 when real work
            # starts. ----
            wsrc = cpool.tile([P, 512], MM, tag="wsrc")
            nc.vector.memset(wsrc[:], 0.0)
            with tc.tile_pool(name="ps_w", bufs=1, space="PSUM") as ps_w:
                wp = ps_w.tile([P, 512], F32, tag="warm")
                for i in range(SPIN):
                    nc.tensor.matmul(
                        wp[:], wsrc[:, 0:P], wsrc[:],
                        start=(i == 0), stop=(i == SPIN - 1),
                    )

            # A-segment chunks of <=512 moving tokens (PSUM bank limit)
            nach = -(-CA // 512)
            ach = []
            off = 0
            for i in range(nach):
                sz = (CA + nach - 1 - i) // nach
                ach.append((off, sz))
                off += sz

            # ---- FFN1: hT = relu(W1^T xg^T + b1) ----
            # W1 streams in 512-col blocks (contiguous 1MB DRAM regions). The
            # first B block rides the sync ring right behind A's so the B
            # stream isn't starved behind xg on the scalar ring; block 0 runs
            # all-A-then-all-B to match its later arrival.
            def ffn1(ps, mb):
                # every block lands as two k-half DMAs: the consuming k-loop
                # unblocks on the first half's semaphore instead of waiting
                # for the whole 1MB block's completion (~2us earlier under the
                # observed completion-semaphore lag)
                w1at = wap.tile([P, KD, MBC], MM, tag="w1at")
                r = slice(mb * P, (mb + 1) * P)
                nc.sync.dma_start(
                    out=w1at[:, 0:half, :], in_=w1a_d[r, 0:half * MBC])
                nc.sync.dma_start(
                    out=w1at[:, half:KD, :], in_=w1a_d[r, half * MBC:])
                if CB:
                    w1bt = wbp.tile([P, KD, MBC], MM, tag="w1bt")
                    eng = nc.sync if mb == 0 else nc.scalar
                    eng.dma_start(
                        out=w1bt[:, 0:half, :], in_=w1b_d[r, 0:half * MBC])
                    eng.dma_start(
                        out=w1bt[:, half:KD, :], in_=w1b_d[r, half * MBC:])

                def a_tile(ml):
                    mt = mb * (MBC // P) + ml
                    pcs = [
                        ps.tile([P, sz], F32, tag=f"psA{ci}",
                                name=f"psA{mt}_{ci}")
                        for ci, (o, sz) in enumerate(ach)
                    ]
                    for k in range(KD):
                        for ci, (o, sz) in enumerate(ach):
                            nc.tensor.matmul(
                                pcs[ci][:],
                                w1at[:, k, ml * P:(ml + 1) * P],
                                xga[:, k, o:o + sz],
                                start=(k == 0), stop=(k == KD - 1),
                            )
                    for ci, (o, sz) in enumerate(ach):
                        nc.vector.tensor_scalar(
                            hta[:, mt, o:o + sz], pcs[ci][:],
                            b1c[:, mt:mt + 1], 0.0, ADD, MAX)

                def b_tile(ml):
                    mt = mb * (MBC // P) + ml
                    psB = ps.tile([P, CB], F32, tag="psB", name=f"psB{mt}")
                    for k in range(KD):
                        nc.tensor.matmul(
                            psB[:],
                            w1bt[:, k, ml * P:(ml + 1) * P],
                            xgb[:, k, :],
                            start=(k == 0), stop=(k == KD - 1),
                        )
                    nc.vector.tensor_scalar(
                        htb[:, mt, :], psB[:],
                        b1c[:, KM + mt:KM + mt + 1], 0.0, ADD, MAX)

                if CB and mb == 0:
                    for ml in range(MBC // P):
                        a_tile(ml)
                    for ml in range(MBC // P):
                        b_tile(ml)
                else:
                    # interleave A/B per k so each LDWEIGHTS hides behind the
                    # previous (longer) A matmul
                    for ml in range(MBC // P):
                        mt = mb * (MBC // P) + ml
                        pcs = [
                            ps.tile([P, sz], F32, tag=f"psA{ci}",
                                    name=f"psA{mt}_{ci}")
                            for ci, (o, sz) in enumerate(ach)
                        ]
                        if CB:
                            psB = ps.tile([P, CB], F32, tag="psB",
                                          name=f"psB{mt}")
                        for k in range(KD):
                            for ci, (o, sz) in enumerate(ach):
                                nc.tensor.matmul(
                                    pcs[ci][:],
                                    w1at[:, k, ml * P:(ml + 1) * P],
                                    xga[:, k, o:o + sz],
                                    start=(k == 0), stop=(k == KD - 1),
                                )
                            if CB:
                                nc.tensor.matmul(
                                    psB[:],
                                    w1bt[:, k, ml * P:(ml + 1) * P],
                                    xgb[:, k, :],
                                    start=(k == 0), stop=(k == KD - 1),
                                )
                        for ci, (o, sz) in enumerate(ach):
                            nc.vector.tensor_scalar(
                                hta[:, mt, o:o + sz], pcs[ci][:],
                                b1c[:, mt:mt + 1], 0.0, ADD, MAX)
                        if CB:
                            nc.vector.tensor_scalar(
                                htb[:, mt, :], psB[:],
                                b1c[:, KM + mt:KM + mt + 1], 0.0, ADD, MAX)

            with tc.tile_pool(name="ps1", bufs=3, space="PSUM") as ps1:
                for mb in range(MLP // MBC):
                    ffn1(ps1, mb)

            # ---- FFN2: yT = W2^T hT ----
            def ffn2(ps, dt):
                w2at = wap.tile([P, KM, P], MM, tag="w2at")
                nc.sync.dma_start(out=w2at[:], in_=w2a_d[dt * P:(dt + 1) * P, :])
                if CB:
                    w2bt = wbp.tile([P, KM, P], MM, tag="w2bt")
                    nc.scalar.dma_start(
                        out=w2bt[:], in_=w2b_d[dt * P:(dt + 1) * P, :])
                pcs = [
                    ps.tile([P, sz], F32, tag=f"ps2A{ci}", name=f"ps2A{dt}_{ci}")
                    for ci, (o, sz) in enumerate(ach)
                ]
                if CB:
                    psB = ps.tile([P, CB], F32, tag="ps2B", name=f"ps2B{dt}")
                for k in range(KM):
                    for ci, (o, sz) in enumerate(ach):
                        nc.tensor.matmul(
                            pcs[ci][:], w2at[:, k, :], hta[:, k, o:o + sz],
                            start=(k == 0), stop=(k == KM - 1),
                        )
                    if CB:
                        nc.tensor.matmul(
                            psB[:], w2bt[:, k, :], htb[:, k, :],
                            start=(k == 0), stop=(k == KM - 1),
                        )
                yat = ypool.tile([P, CA], MM, tag="yat")
                # y rides the (otherwise idle) gpsimd SWDGE queue; the last
                # d-tile goes out in halves on sync+scalar (empty by then) so
                # the final cast/issue/transfer chains overlap
                if dt == D // P - 1:
                    hh = CA // 2
                    nc.vector.tensor_copy(yat[:, 0:hh], pcs[0][:, 0:hh])
                    nc.sync.dma_start(
                        out=ya_d[dt * P:(dt + 1) * P, 0:hh], in_=yat[:, 0:hh])
                    for ci, (o, sz) in enumerate(ach):
                        oo = max(o, hh)
                        if o + sz > hh:
                            nc.vector.tensor_copy(
                                yat[:, oo:o + sz], pcs[ci][:, oo - o:sz])
                    nc.scalar.dma_start(
                        out=ya_d[dt * P:(dt + 1) * P, hh:], in_=yat[:, hh:])
                else:
                    for ci, (o, sz) in enumerate(ach):
                        nc.vector.tensor_copy(yat[:, o:o + sz], pcs[ci][:])
                    nc.gpsimd.dma_start(
                        out=ya_d[dt * P:(dt + 1) * P, :], in_=yat[:])
                if CB:
                    ybt = ypool.tile([P, CB], MM, tag="ybt")
                    nc.vector.tensor_copy(ybt[:], psB[:])
                    eng_b = nc.scalar if dt == D // P - 1 else nc.gpsimd
                    eng_b.dma_start(out=yb_d[dt * P:(dt + 1) * P, :], in_=ybt[:])

            with tc.tile_pool(name="ps2", bufs=3, space="PSUM") as ps2:
                for dt in range(D // P):
                    ffn2(ps2, dt)
    nc.compile()
    return nc


def _choose_capacity(counts):
    """Pick (CA, CB) so all spill chunks fit in the 8 per-core B slots."""
    force = os.environ.get("BASS_MOE_FORCE")
    if force:
        CA, CB = (int(v) for v in force.split(","))
        if CA >= max(counts) or (
                CB and sum(-(-max(c - CA, 0) // CB) for c in counts) <= E):
            return CA, CB
    for CA, CB in [(464, 72), (512, 96), (576, 128), (640, 192)]:
        need = sum(-(-max(c - CA, 0) // CB) for c in counts)
        if need <= E:
            return CA, CB
    return max(counts), 0


def _pack_weights(W1, W2, b1):
    """Host-side DRAM layouts: every device DMA is a contiguous [128,N] block."""
    w1x = np.ascontiguousarray(
        W1.reshape(E, KD, P, MLP // MBC, MBC).transpose(0, 3, 2, 1, 4)
        .reshape(E, D, MLP)).astype(NP_MM)
    w2x = np.ascontiguousarray(
        W2.reshape(E, KM, P, KD, P).transpose(0, 3, 2, 1, 4)
        .reshape(E, D, MLP)).astype(NP_MM)
    b1x = np.ascontiguousarray(b1.reshape(E, KM, P).transpose(0, 2, 1))
    return w1x, w2x, b1x


def _xg_pack(x_rows, C):
    """[n,D] tokens -> [128, KD*C] bf16 (k-major, zero-padded)."""
    out = np.zeros((P, KD * C), NP_MM)
    n = x_rows.shape[0]
    if n:
        xt = x_rows.T.astype(NP_MM).reshape(KD, P, n).transpose(1, 0, 2)
        out.reshape(P, KD, C)[:, :, :n] = xt
    return out


def kernel(x, w_gate, b_gate, W1, b1, W2, b2):
    x = np.ascontiguousarray(x, np.float32)
    w_gate = np.ascontiguousarray(w_gate, np.float32)
    b_gate = np.ascontiguousarray(b_gate, np.float32)
    W1 = np.ascontiguousarray(W1, np.float32)
    b1 = np.ascontiguousarray(b1, np.float32)
    W2 = np.ascontiguousarray(W2, np.float32)
    b2 = np.ascontiguousarray(b2, np.float32)

    x_flat = x.reshape(N_TOK, D)
    logits = x_flat @ w_gate + b_gate
    idx = logits.argmax(-1)
    lmax = logits.max(-1, keepdims=True)
    ee = np.exp(logits - lmax)
    p = (ee.max(-1) / ee.sum(-1)).astype(np.float32)

    ids = [np.nonzero(idx == e)[0] for e in range(E)]
    counts = [len(i) for i in ids]
    CA, CB = _choose_capacity(counts)

    wkey = (W1.ctypes.data, W2.ctypes.data, b1.ctypes.data)
    if _host_cache.get("wkey") != wkey:
        _host_cache["wkey"] = wkey
        _host_cache["packed"] = _pack_weights(W1, W2, b1)
    w1x, w2x, b1x = _host_cache["packed"]

    # spill chunks: tokens beyond CA per expert, split into <=CB pieces
    chunks = []
    for e in range(E):
        rem = ids[e][CA:]
        for s in range(0, len(rem), CB if CB else 1):
            if CB:
                chunks.append((e, rem[s:s + CB]))
    assert len(chunks) <= E, (counts, CA, CB)

    in_maps = []
    b_assign = []
    for c in range(E):
        a_ids = ids[c][:CA]
        m = {
            "xga": _xg_pack(x_flat[a_ids], CA),
            "w1a": w1x[c], "w2a": w2x[c],
        }
        if c < len(chunks):
            eb, b_ids = chunks[c]
        else:
            eb, b_ids = c, np.empty(0, np.int64)
        b_assign.append((eb, b_ids))
        b1cm = np.concatenate([b1x[c], b1x[eb]], axis=1).astype(np.float32)
        m["b1c"] = np.ascontiguousarray(b1cm)
        if CB:
            m["xgb"] = _xg_pack(x_flat[b_ids], CB)
            m["w1b"] = w1x[eb]
            m["w2b"] = w2x[eb]
        in_maps.append(m)

    key = (CA, CB)
    if key not in _cached_nc:
        _cached_nc[key] = build_nc(CA, CB)
    nc = _cached_nc[key]

    res = bass_utils.run_bass_kernel_spmd(nc, in_maps, list(range(E)))

    out_flat = np.empty((N_TOK, D), np.float32)
    for c in range(E):
        a_ids = ids[c][:CA]
        ya = res.results[c]["ya"].astype(np.float32)  # [D, CA]
        out_flat[a_ids] = (ya.T[:len(a_ids)] + b2[c]) * p[a_ids, None]
        eb, b_ids = b_assign[c]
        if len(b_ids):
            yb = res.results[c]["yb"].astype(np.float32)
            out_flat[b_ids] = (yb.T[:len(b_ids)] + b2[eb]) * p[b_ids, None]
    return out_flat.reshape(B, T, D)
